# revision 1
# baseline (speedup 1.0000x reference)
"""ConvCapsuleLayer Trainium2 kernel: 5x5 conv (16->128ch) + 3-iter dynamic routing.

Sharding: H (256) split into 8 bands of 32 rows (halo 2 via host padding).
Each core computes conv + routing for its band; outputs concat along H.

fp16 data / fp32 accumulation; folded-factor routing (squash factor applied
after the PE reduction, so parent is never materialized except at the end).
"""
import sys
sys.path.insert(0, "/opt/trn_rl_repo")
import numpy as np

import concourse.bass as bass
import concourse.mybir as mybir
import concourse.tile as tile
import concourse.bacc as bacc_mod
from concourse.bass_utils import run_bass_kernel_spmd

dt = mybir.dt
F16 = dt.float16
F32 = dt.float32
NPF16 = np.float16
AF = mybir.ActivationFunctionType

B, NC, LC, H, Wd = 4, 4, 16, 256, 256
NP, LP = 8, 16
NCORES = 8
HB = H // NCORES          # 32 rows per core
RG = 4                    # out-rows per row-group
NG = HB // RG             # 8 row-groups
PIX = RG * Wd             # 1024
HPIX = 512
WPAD = Wd + 4             # 260

_nc_cache = {}


def build_nc():
    import os as _os
    DBG = _os.environ.get("KDEBUG") == "1"
    nc = bacc_mod.Bacc()

    xs = nc.declare_dram_parameter("xs", [B, NC, LC, HB + 4, WPAD], F16, isOutput=False)
    xm = nc.declare_dram_parameter("xm", [B, LC, HB + 4, WPAD], F16, isOutput=False)
    wt = nc.declare_dram_parameter("wt", [80, 5, 128], F16, isOutput=False)
    selnp = nc.declare_dram_parameter("selnp", [128, 32], F16, isOutput=False)
    selb = nc.declare_dram_parameter("selb", [128, 4, 32], F16, isOutput=False)
    sumsel = nc.declare_dram_parameter("sumsel", [128, 16], F32, isOutput=False)
    csel = nc.declare_dram_parameter("csel", [128, 16, 128], F16, isOutput=False)
    y = nc.declare_dram_parameter("y", [B, 128, HB, Wd], F32, isOutput=True)
    if DBG:
        dv = nc.declare_dram_parameter("d_votes", [128, PIX], F32, isOutput=True)
        dp0 = nc.declare_dram_parameter("d_pb0", [128, PIX], F32, isOutput=True)
        dsq = nc.declare_dram_parameter("d_sq", [128, PIX], F32, isOutput=True)
        dfac = nc.declare_dram_parameter("d_fac", [128, PIX], F32, isOutput=True)
        du = nc.declare_dram_parameter("d_u", [128, PIX], F32, isOutput=True)
        dsims = nc.declare_dram_parameter("d_sims", [128, PIX], F32, isOutput=True)
        dcall = nc.declare_dram_parameter("d_call", [128, PIX], F32, isOutput=True)
        dcbc = nc.declare_dram_parameter("d_cbc", [128, PIX], F32, isOutput=True)
        dpb1 = nc.declare_dram_parameter("d_pb1", [128, PIX], F32, isOutput=True)

    with tile.TileContext(nc) as tc:
        with (
            tc.tile_pool(name="const", bufs=1) as cpool,
            tc.tile_pool(name="xstk", bufs=7) as xpool,
            tc.tile_pool(name="votes", bufs=16 if DBG else 22) as vpool,
            tc.tile_pool(name="pb", bufs=12) as pbpool,
            tc.tile_pool(name="sqs", bufs=7) as sqpool,
            tc.tile_pool(name="f16w", bufs=8) as fpool,
            tc.tile_pool(name="adds", bufs=6) as apool,
            tc.tile_pool(name="sp1", bufs=1) as sp1,
            tc.tile_pool(name="sp2", bufs=1 if DBG else 2) as sp2,
            tc.tile_pool(name="vps", bufs=2, space="PSUM") as vps,
            tc.tile_pool(name="ups", bufs=2, space="PSUM") as ups,
            tc.tile_pool(name="cbps", bufs=2, space="PSUM") as cbps,
            tc.tile_pool(name="sps", bufs=2, space="PSUM") as sps,
        ):
            wt_t = cpool.tile([80, 5, 128], F16)
            nc.sync.dma_start(wt_t[:], wt[:])
            selnp_t = cpool.tile([128, 32], F16)
            nc.sync.dma_start(selnp_t[:], selnp[:])
            selb_t = cpool.tile([128, 4, 32], F16)
            nc.sync.dma_start(selb_t[:], selb[:])
            sumsel_t = cpool.tile([128, 16], F32)
            nc.sync.dma_start(sumsel_t[:], sumsel[:])
            csel_t = cpool.tile([128, 16, 128], F16)
            nc.sync.dma_start(csel_t[:], csel[:])
            bias1 = cpool.tile([128, 1], F32)
            nc.gpsimd.memset(bias1[:], 1.0 + 1e-4)
            bias_e = cpool.tile([128, 1], F32)
            nc.gpsimd.memset(bias_e[:], 1e-4)

            for g in range(1 if DBG else NG):
                s0 = g * RG
                votes = {}
                pb16 = {}
                sqs = {}
                for b in range(B):
                    stk = []
                    for n in range(NC):
                        t = xpool.tile([80, RG, WPAD], F16, tag="xstk")
                        src = xs[b, n, :, s0: s0 + RG, :]
                        src.ap = [[WPAD, 5]] + src.ap   # overlapping ky dim
                        nc.sync.dma_start(t[:], src)
                        stk.append(t)
                    tm = xpool.tile([80, RG, WPAD], F16, tag="xstk")
                    srcm = xm[b, :, s0: s0 + RG, :]
                    srcm.ap = [[WPAD, 5]] + srcm.ap
                    nc.sync.dma_start(tm[:], srcm)

                    for n in range(NC):
                        vt = vpool.tile([128, PIX], F16, tag="votes")
                        ph = [vps.tile([128, HPIX], F32, tag="vps",
                                       name=f"vps{g}_{b}_{n}_{_h}") for _h in range(2)]
                        for kx in range(5):
                            for hh in range(2):
                                nc.tensor.matmul(
                                    ph[hh][:], wt_t[:, kx, :],
                                    stk[n][:, 2 * hh: 2 * hh + 2, kx: kx + Wd],
                                    start=(kx == 0), stop=(kx == 4))
                        for hh in range(2):
                            nc.scalar.copy(vt[:, hh * HPIX:(hh + 1) * HPIX], ph[hh][:])
                        votes[(b, n)] = vt
                    # mean-x conv -> parent_bs of iter 0 (xm pre-divided by 8)
                    v0 = pbpool.tile([128, PIX], F16, tag="pb")
                    sq0 = sqpool.tile([128, PIX], F16, tag="sqs")
                    ph = [vps.tile([128, HPIX], F32, tag="vps",
                                   name=f"vpsm{g}_{b}_{_h}") for _h in range(2)]
                    for kx in range(5):
                        for hh in range(2):
                            nc.tensor.matmul(
                                ph[hh][:], wt_t[:, kx, :],
                                tm[:, 2 * hh: 2 * hh + 2, kx: kx + Wd],
                                start=(kx == 0), stop=(kx == 4))
                    for hh in range(2):
                        sl = slice(hh * HPIX, (hh + 1) * HPIX)
                        nc.scalar.copy(v0[:, sl], ph[hh][:])
                        nc.scalar.square(sq0[:, sl], ph[hh][:])
                    pb16[b] = v0
                    sqs[b] = sq0
                    if DBG and g == 0 and b == 0:
                        dvt = sp1.tile([128, PIX], F32, tag="dvt")
                        nc.vector.tensor_copy(dvt[:], votes[(0, 0)][:])
                        nc.sync.dma_start(dv[:], dvt[:])
                        dvt2 = sp1.tile([128, PIX], F32, tag="dvt2")
                        nc.vector.tensor_copy(dvt2[:], v0[:])
                        nc.sync.dma_start(dp0[:], dvt2[:])

                sims = sp2.tile([128, PIX], F32, tag="sims")

                for it in range(3):
                    if it > 0:
                        for b in range(B):
                            sq = sqpool.tile([128, PIX], F16, tag="sqs")
                            nc.vector.tensor_mul(sq[:], pb16[b][:], pb16[b][:])
                            sqs[b] = sq
                    # sq_all rows b*32+np via col-tiled selector mms
                    sqh = []
                    for hh in range(2):
                        sqp = cbps.tile([128, HPIX], F32, tag="cbps", name=f"sq{g}_{it}_{hh}")
                        sl = slice(hh * HPIX, (hh + 1) * HPIX)
                        for b in range(B):
                            nc.tensor.matmul(
                                sqp[32 * b:32 * (b + 1), :], selnp_t[:],
                                sqs[b][:, sl], start=True, stop=True,
                                tile_position=(0, 32 * b))
                        sqh.append(sqp)
                    sr = sp1.tile([128, PIX], F32, tag="sr")
                    dd = sp1.tile([128, PIX], F32, tag="dd")
                    for hh in range(2):
                        sl = slice(hh * HPIX, (hh + 1) * HPIX)
                        nc.scalar.activation(sr[:, sl], sqh[hh][:], AF.Sqrt)
                        nc.vector.tensor_scalar_add(dd[:, sl], sqh[hh][:], 1.0 + 1e-4)
                    rd = sp1.tile([128, PIX], F32, tag="rd")
                    nc.vector.reciprocal_approx_fast(rd[:], dd[:])
                    fac = sp2.tile([128, PIX], F32, tag="fac")
                    nc.vector.tensor_mul(fac[:], sr[:], rd[:])
                    if DBG and g == 0 and it == 0:
                        dsqt = sp1.tile([128, PIX], F32, tag="dsqt")
                        for hh in range(2):
                            nc.vector.tensor_copy(dsqt[:, hh*HPIX:(hh+1)*HPIX], sqh[hh][:])
                        nc.sync.dma_start(dsq[:], dsqt[:])
                        nc.sync.dma_start(dfac[:], fac[:])

                    if it < 2:
                        uh = [ups.tile([128, HPIX], F32, tag="ups", name=f"uh{it}_{_h}") for _h in range(2)]
                        for b in range(B):
                            for n in range(NC):
                                r = fpool.tile([128, PIX], F16, tag="f16w")
                                nc.vector.tensor_mul(r[:], votes[(b, n)][:], pb16[b][:])
                                for hh in range(2):
                                    sl = slice(hh * HPIX, (hh + 1) * HPIX)
                                    nc.tensor.matmul(
                                        uh[hh][32 * n:32 * (n + 1), :],
                                        selb_t[:, b, :], r[:, sl],
                                        start=(b == 0), stop=(b == B - 1),
                                        tile_position=(0, 32 * n))
                        # fac_rep rows nc*32+b*8+np <- fac rows b*32+np
                        facr = sp2.tile([128, PIX], F32, tag="facr")
                        for n in range(NC):
                            for b in range(B):
                                nc.sync.dma_start(
                                    facr[n * 32 + b * 8: n * 32 + b * 8 + 8, :],
                                    fac[b * 32: b * 32 + 8, :])
                        tgt = sims if it == 0 else sp2.tile([128, PIX], F32, tag="fu", name=f"fu{it}")
                        for hh in range(2):
                            sl = slice(hh * HPIX, (hh + 1) * HPIX)
                            nc.vector.tensor_mul(tgt[:, sl], facr[:, sl], uh[hh][:])
                        if it > 0:
                            nc.vector.tensor_add(sims[:], sims[:], tgt[:])
                        if DBG and g == 0 and it == 0:
                            dut = sp1.tile([128, PIX], F32, tag="dut")
                            for hh in range(2):
                                nc.vector.tensor_copy(dut[:, hh*HPIX:(hh+1)*HPIX], uh[hh][:])
                            nc.sync.dma_start(du[:], dut[:])
                            nc.sync.dma_start(dsims[:], sims[:])

                        e = sp1.tile([128, PIX], F32, tag="e")
                        nc.scalar.activation(e[:], sims[:], AF.Exp, bias=bias_e[:])
                        rs = sp2.tile([16, PIX], F32, tag="rs")
                        for hh in range(2):
                            sl = slice(hh * HPIX, (hh + 1) * HPIX)
                            sp_ = sps.tile([16, HPIX], F32, tag="sps")
                            nc.tensor.matmul(sp_[:], sumsel_t[:], e[:, sl],
                                             start=True, stop=True)
                            nc.vector.reciprocal_approx_fast(rs[:, sl], sp_[:])
                        rsb = sp1.tile([128, PIX], F32, tag="rsb")
                        rsb_r = rsb.rearrange("(m p) f -> p m f", m=16)
                        for j in range(8):
                            nc.sync.dma_start(rsb_r[j], rs[:])
                        call = sp2.tile([128, PIX], F16, tag="call")
                        nc.vector.tensor_mul(call[:], e[:], rsb[:])

                        for b in range(B):
                            pb = pbpool.tile([128, PIX], F16, tag="pb")
                            t1 = apool.tile([128, PIX], F16, tag="adds")
                            t2 = apool.tile([128, PIX], F16, tag="adds")
                            prev_q = None
                            for n in range(NC):
                                cbc = fpool.tile([128, PIX], F16, tag="f16w")
                                for hh in range(2):
                                    sl = slice(hh * HPIX, (hh + 1) * HPIX)
                                    cps = cbps.tile([128, HPIX], F32, tag="cbps")
                                    nc.tensor.matmul(cps[:], csel_t[:, b * 4 + n, :],
                                                     call[:, sl], start=True, stop=True)
                                    nc.scalar.copy(cbc[:, sl], cps[:])
                                q = fpool.tile([128, PIX], F16, tag="f16w")
                                nc.vector.tensor_mul(q[:], cbc[:], votes[(b, n)][:])
                                if n == 1:
                                    nc.vector.tensor_add(t1[:], prev_q[:], q[:])
                                elif n == 3:
                                    nc.vector.tensor_add(t2[:], prev_q[:], q[:])
                                prev_q = q
                            nc.vector.tensor_add(pb[:], t1[:], t2[:])
                            pb16[b] = pb
                            if DBG and g == 0 and it == 0 and b == 0:
                                dct = sp1.tile([128, PIX], F32, tag="dct")
                                nc.vector.tensor_copy(dct[:], call[:])
                                nc.sync.dma_start(dcall[:], dct[:])
                                dcb = sp1.tile([128, PIX], F32, tag="dcb")
                                nc.vector.tensor_copy(dcb[:], cbc[:])
                                nc.sync.dma_start(dcbc[:], dcb[:])
                                dpt = sp1.tile([128, PIX], F32, tag="dpt")
                                nc.vector.tensor_copy(dpt[:], pb[:])
                                nc.sync.dma_start(dpb1[:], dpt[:])
                    else:
                        fac16 = sp1.tile([128, PIX], F16, tag="fac16")
                        nc.scalar.copy(fac16[:], fac[:])
                        for b in range(B):
                            fbc = sp1.tile([128, PIX], F16, tag="fbc")
                            nc.sync.dma_start(fbc[0:8, :],
                                              fac16[b * 32: b * 32 + 8, :])
                            for k in (8, 16, 32, 64):
                                nc.sync.dma_start(fbc[k:2 * k, :], fbc[0:k, :])
                            out = sp2.tile([128, PIX], F32, tag="outt")
                            nc.vector.tensor_mul(out[:], fbc[:], pb16[b][:])
                            nc.sync.dma_start(
                                y[b, :, s0:s0 + RG, :].rearrange(
                                    "(p l) r w -> l p r w", p=8, l=16),
                                out.rearrange("p (r w) -> p r w", r=RG))

    nc.compile()
    return nc


def _prep_inputs(x, W):
    x = np.asarray(x, np.float32)
    W = np.asarray(W, np.float32)
    # oc' = lp*8+np ordering of output channels
    perm = np.zeros(128, np.int64)
    for np_ in range(8):
        for lp in range(16):
            perm[lp * 8 + np_] = np_ * 16 + lp
    wt = np.zeros((80, 5, 128), np.float32)
    for kx in range(5):
        for ky in range(5):
            wt[ky * 16:(ky + 1) * 16, kx, :] = W[perm, :, ky, kx].T
    wt = wt.astype(NPF16)

    csel = np.zeros((128, 16, 128), NPF16)
    for b in range(4):
        for n in range(4):
            for m in range(128):
                csel[n * 32 + b * 8 + (m % 8), b * 4 + n, m] = 1.0

    selnp = np.zeros((128, 32), NPF16)
    for p in range(128):
        selnp[p, p % 8] = 1.0
    selb = np.zeros((128, 4, 32), NPF16)
    for b in range(4):
        for p in range(128):
            selb[p, b, b * 8 + p % 8] = 1.0
    sumsel = np.zeros((128, 16), np.float32)
    for p in range(128):
        sumsel[p, (p // 32) * 4 + (p % 32) // 8] = 1.0

    xp = np.zeros((B, NC, LC, H + 4, WPAD), np.float32)
    xp[:, :, :, 2:-2, 2:-2] = x
    xmf = xp.sum(axis=1) / 8.0
    xp16 = xp.astype(NPF16)
    xm16 = xmf.astype(NPF16)

    in_maps = []
    for k in range(NCORES):
        r0 = k * HB
        in_maps.append({
            "xs": np.ascontiguousarray(xp16[:, :, :, r0:r0 + HB + 4, :]),
            "xm": np.ascontiguousarray(xm16[:, :, r0:r0 + HB + 4, :]),
            "wt": wt, "selnp": selnp, "selb": selb, "sumsel": sumsel,
            "csel": csel,
        })
    return in_maps


def kernel(x, W):
    if "nc" not in _nc_cache:
        _nc_cache["nc"] = build_nc()
    nc = _nc_cache["nc"]
    in_maps = _prep_inputs(x, W)
    res = run_bass_kernel_spmd(nc, in_maps, list(range(NCORES))).results
    out = np.concatenate([r["y"] for r in res], axis=2)
    return out.reshape(B, NP, LP, H, Wd).astype(np.float32)



# revision 2
# speedup vs baseline: 2.5665x; 2.5665x over previous
"""ConvCapsuleLayer Trainium2 kernel: 5x5 conv (16->128ch) + 3-iter dynamic routing.

Sharding: H (256) split into 8 bands of 32 rows (halo 2 via host padding).
Each core computes conv + routing for its band; outputs concat along H.

v2: the axon tunnel (~45MB/s up, ~38MB/s down, half-duplex) dominates wall
time, so minimize bytes on the wire and per-call dispatch overhead:
  - y returned as fp16 (halves d2h)
  - iter-0 parent computed on device from the vote tiles (drops the xm input)
  - cached jit'd shard_map runner (run_bass_kernel_spmd rebuilds the jit every
    call -> re-trace + XLA compile each time); donated output buffers are
    created on-device (zeros were previously shipped over the tunnel)
"""
import sys
sys.path.insert(0, "/opt/trn_rl_repo")
import numpy as np

import concourse.bass as bass
import concourse.mybir as mybir
import concourse.tile as tile
import concourse.bacc as bacc_mod
from concourse._compat import axon_active

dt = mybir.dt
F16 = dt.float16
F32 = dt.float32
F8 = dt.float8e4
NPF16 = np.float16
AF = mybir.ActivationFunctionType

B, NC, LC, H, Wd = 4, 4, 16, 256, 256
NP, LP = 8, 16
NCORES = 8
HB = H // NCORES          # 32 rows per core
RG = 4                    # out-rows per row-group
NG = HB // RG             # 8 row-groups
PIX = RG * Wd             # 1024
HPIX = 512
WPAD = Wd + 4             # 260

USE_FP8X = False          # ship x as fp8-e4m3, upcast on device

_cache = {}


def build_nc():
    nc = bacc_mod.Bacc()

    XDT = F8 if USE_FP8X else F16
    xs = nc.declare_dram_parameter("xs", [B, NC, LC, HB + 4, WPAD], XDT, isOutput=False)
    wt = nc.declare_dram_parameter("wt", [80, 5, 128], F16, isOutput=False)
    selnp = nc.declare_dram_parameter("selnp", [128, 32], F16, isOutput=False)
    selb = nc.declare_dram_parameter("selb", [128, 4, 32], F16, isOutput=False)
    sumsel = nc.declare_dram_parameter("sumsel", [128, 16], F32, isOutput=False)
    csel = nc.declare_dram_parameter("csel", [128, 16, 128], F16, isOutput=False)
    y = nc.declare_dram_parameter("y", [B, 128, HB, Wd], F16, isOutput=True)

    with tile.TileContext(nc) as tc:
        with (
            tc.tile_pool(name="const", bufs=1) as cpool,
            tc.tile_pool(name="xstk", bufs=7) as xpool,
            tc.tile_pool(name="x8", bufs=3) as x8pool,
            tc.tile_pool(name="votes", bufs=22) as vpool,
            tc.tile_pool(name="pb", bufs=12) as pbpool,
            tc.tile_pool(name="sqs", bufs=7) as sqpool,
            tc.tile_pool(name="f16w", bufs=8) as fpool,
            tc.tile_pool(name="adds", bufs=6) as apool,
            tc.tile_pool(name="sp1", bufs=1) as sp1,
            tc.tile_pool(name="sp2", bufs=2) as sp2,
            tc.tile_pool(name="vps", bufs=2, space="PSUM") as vps,
            tc.tile_pool(name="ups", bufs=2, space="PSUM") as ups,
            tc.tile_pool(name="cbps", bufs=2, space="PSUM") as cbps,
            tc.tile_pool(name="sps", bufs=2, space="PSUM") as sps,
        ):
            wt_t = cpool.tile([80, 5, 128], F16)
            nc.sync.dma_start(wt_t[:], wt[:])
            selnp_t = cpool.tile([128, 32], F16)
            nc.sync.dma_start(selnp_t[:], selnp[:])
            selb_t = cpool.tile([128, 4, 32], F16)
            nc.sync.dma_start(selb_t[:], selb[:])
            sumsel_t = cpool.tile([128, 16], F32)
            nc.sync.dma_start(sumsel_t[:], sumsel[:])
            csel_t = cpool.tile([128, 16, 128], F16)
            nc.sync.dma_start(csel_t[:], csel[:])
            bias_e = cpool.tile([128, 1], F32)
            nc.gpsimd.memset(bias_e[:], 1e-4)

            for g in range(NG):
                s0 = g * RG
                votes = {}
                pb16 = {}
                sqs = {}
                for b in range(B):
                    stk = []
                    for n in range(NC):
                        if USE_FP8X:
                            t8 = x8pool.tile([80, RG, WPAD], F8, tag="x8")
                            src = xs[b, n, :, s0: s0 + RG, :]
                            src.ap = [[WPAD, 5]] + src.ap   # overlapping ky dim
                            nc.sync.dma_start(t8[:], src)
                            t = xpool.tile([80, RG, WPAD], F16, tag="xstk")
                            nc.scalar.copy(t[:], t8[:])
                        else:
                            t = xpool.tile([80, RG, WPAD], F16, tag="xstk")
                            src = xs[b, n, :, s0: s0 + RG, :]
                            src.ap = [[WPAD, 5]] + src.ap   # overlapping ky dim
                            nc.sync.dma_start(t[:], src)
                        stk.append(t)

                    for n in range(NC):
                        vt = vpool.tile([128, PIX], F16, tag="votes")
                        ph = [vps.tile([128, HPIX], F32, tag="vps",
                                       name=f"vps{g}_{b}_{n}_{_h}") for _h in range(2)]
                        for kx in range(5):
                            for hh in range(2):
                                nc.tensor.matmul(
                                    ph[hh][:], wt_t[:, kx, :],
                                    stk[n][:, 2 * hh: 2 * hh + 2, kx: kx + Wd],
                                    start=(kx == 0), stop=(kx == 4))
                        for hh in range(2):
                            nc.scalar.copy(vt[:, hh * HPIX:(hh + 1) * HPIX], ph[hh][:])
                        votes[(b, n)] = vt
                    # iter-0 parent_bs = (sum_nc votes)/8 (softmax(0) over NP=8)
                    t1 = apool.tile([128, PIX], F16, tag="adds")
                    t2 = apool.tile([128, PIX], F16, tag="adds")
                    nc.vector.tensor_add(t1[:], votes[(b, 0)][:], votes[(b, 1)][:])
                    nc.vector.tensor_add(t2[:], votes[(b, 2)][:], votes[(b, 3)][:])
                    nc.vector.tensor_add(t1[:], t1[:], t2[:])
                    v0 = pbpool.tile([128, PIX], F16, tag="pb")
                    sq0 = sqpool.tile([128, PIX], F16, tag="sqs")
                    nc.scalar.mul(v0[:], t1[:], 0.125)
                    nc.scalar.activation(sq0[:], t1[:], AF.Square, scale=0.125)
                    pb16[b] = v0
                    sqs[b] = sq0

                sims = sp2.tile([128, PIX], F32, tag="sims")

                for it in range(3):
                    if it > 0:
                        for b in range(B):
                            sq = sqpool.tile([128, PIX], F16, tag="sqs")
                            nc.vector.tensor_mul(sq[:], pb16[b][:], pb16[b][:])
                            sqs[b] = sq
                    # sq_all rows b*32+np via col-tiled selector mms
                    sqh = []
                    for hh in range(2):
                        sqp = cbps.tile([128, HPIX], F32, tag="cbps", name=f"sq{g}_{it}_{hh}")
                        sl = slice(hh * HPIX, (hh + 1) * HPIX)
                        for b in range(B):
                            nc.tensor.matmul(
                                sqp[32 * b:32 * (b + 1), :], selnp_t[:],
                                sqs[b][:, sl], start=True, stop=True,
                                tile_position=(0, 32 * b))
                        sqh.append(sqp)
                    sr = sp1.tile([128, PIX], F32, tag="sr")
                    dd = sp1.tile([128, PIX], F32, tag="dd")
                    for hh in range(2):
                        sl = slice(hh * HPIX, (hh + 1) * HPIX)
                        nc.scalar.activation(sr[:, sl], sqh[hh][:], AF.Sqrt)
                        nc.vector.tensor_scalar_add(dd[:, sl], sqh[hh][:], 1.0 + 1e-4)
                    rd = sp1.tile([128, PIX], F32, tag="rd")
                    nc.vector.reciprocal_approx_fast(rd[:], dd[:])
                    fac = sp2.tile([128, PIX], F32, tag="fac")
                    nc.vector.tensor_mul(fac[:], sr[:], rd[:])

                    if it < 2:
                        uh = [ups.tile([128, HPIX], F32, tag="ups", name=f"uh{it}_{_h}") for _h in range(2)]
                        for b in range(B):
                            for n in range(NC):
                                r = fpool.tile([128, PIX], F16, tag="f16w")
                                nc.vector.tensor_mul(r[:], votes[(b, n)][:], pb16[b][:])
                                for hh in range(2):
                                    sl = slice(hh * HPIX, (hh + 1) * HPIX)
                                    nc.tensor.matmul(
                                        uh[hh][32 * n:32 * (n + 1), :],
                                        selb_t[:, b, :], r[:, sl],
                                        start=(b == 0), stop=(b == B - 1),
                                        tile_position=(0, 32 * n))
                        # fac_rep rows nc*32+b*8+np <- fac rows b*32+np
                        facr = sp2.tile([128, PIX], F32, tag="facr")
                        for n in range(NC):
                            for b in range(B):
                                nc.sync.dma_start(
                                    facr[n * 32 + b * 8: n * 32 + b * 8 + 8, :],
                                    fac[b * 32: b * 32 + 8, :])
                        tgt = sims if it == 0 else sp2.tile([128, PIX], F32, tag="fu", name=f"fu{it}")
                        for hh in range(2):
                            sl = slice(hh * HPIX, (hh + 1) * HPIX)
                            nc.vector.tensor_mul(tgt[:, sl], facr[:, sl], uh[hh][:])
                        if it > 0:
                            nc.vector.tensor_add(sims[:], sims[:], tgt[:])

                        e = sp1.tile([128, PIX], F32, tag="e")
                        nc.scalar.activation(e[:], sims[:], AF.Exp, bias=bias_e[:])
                        rs = sp2.tile([16, PIX], F32, tag="rs")
                        for hh in range(2):
                            sl = slice(hh * HPIX, (hh + 1) * HPIX)
                            sp_ = sps.tile([16, HPIX], F32, tag="sps")
                            nc.tensor.matmul(sp_[:], sumsel_t[:], e[:, sl],
                                             start=True, stop=True)
                            nc.vector.reciprocal_approx_fast(rs[:, sl], sp_[:])
                        rsb = sp1.tile([128, PIX], F32, tag="rsb")
                        rsb_r = rsb.rearrange("(m p) f -> p m f", m=16)
                        for j in range(8):
                            nc.sync.dma_start(rsb_r[j], rs[:])
                        call = sp2.tile([128, PIX], F16, tag="call")
                        nc.vector.tensor_mul(call[:], e[:], rsb[:])

                        for b in range(B):
                            pb = pbpool.tile([128, PIX], F16, tag="pb")
                            t1 = apool.tile([128, PIX], F16, tag="adds")
                            t2 = apool.tile([128, PIX], F16, tag="adds")
                            prev_q = None
                            for n in range(NC):
                                cbc = fpool.tile([128, PIX], F16, tag="f16w")
                                for hh in range(2):
                                    sl = slice(hh * HPIX, (hh + 1) * HPIX)
                                    cps = cbps.tile([128, HPIX], F32, tag="cbps")
                                    nc.tensor.matmul(cps[:], csel_t[:, b * 4 + n, :],
                                                     call[:, sl], start=True, stop=True)
                                    nc.scalar.copy(cbc[:, sl], cps[:])
                                q = fpool.tile([128, PIX], F16, tag="f16w")
                                nc.vector.tensor_mul(q[:], cbc[:], votes[(b, n)][:])
                                if n == 1:
                                    nc.vector.tensor_add(t1[:], prev_q[:], q[:])
                                elif n == 3:
                                    nc.vector.tensor_add(t2[:], prev_q[:], q[:])
                                prev_q = q
                            nc.vector.tensor_add(pb[:], t1[:], t2[:])
                            pb16[b] = pb
                    else:
                        fac16 = sp1.tile([128, PIX], F16, tag="fac16")
                        nc.scalar.copy(fac16[:], fac[:])
                        for b in range(B):
                            fbc = sp1.tile([128, PIX], F16, tag="fbc")
                            nc.sync.dma_start(fbc[0:8, :],
                                              fac16[b * 32: b * 32 + 8, :])
                            for k in (8, 16, 32, 64):
                                nc.sync.dma_start(fbc[k:2 * k, :], fbc[0:k, :])
                            out = sp2.tile([128, PIX], F16, tag="outt")
                            nc.vector.tensor_mul(out[:], fbc[:], pb16[b][:])
                            nc.sync.dma_start(
                                y[b, :, s0:s0 + RG, :].rearrange(
                                    "(p l) r w -> l p r w", p=8, l=16),
                                out.rearrange("p (r w) -> p r w", r=RG))

    nc.compile()
    return nc


def _prep_inputs(x, W):
    x = np.asarray(x, np.float32)
    W = np.asarray(W, np.float32)
    # oc' = lp*8+np ordering of output channels
    perm = np.zeros(128, np.int64)
    for np_ in range(8):
        for lp in range(16):
            perm[lp * 8 + np_] = np_ * 16 + lp
    wt = np.zeros((80, 5, 128), np.float32)
    for kx in range(5):
        for ky in range(5):
            wt[ky * 16:(ky + 1) * 16, kx, :] = W[perm, :, ky, kx].T
    wt = wt.astype(NPF16)

    csel = np.zeros((128, 16, 128), NPF16)
    for b in range(4):
        for n in range(4):
            for m in range(128):
                csel[n * 32 + b * 8 + (m % 8), b * 4 + n, m] = 1.0

    selnp = np.zeros((128, 32), NPF16)
    for p in range(128):
        selnp[p, p % 8] = 1.0
    selb = np.zeros((128, 4, 32), NPF16)
    for b in range(4):
        for p in range(128):
            selb[p, b, b * 8 + p % 8] = 1.0
    sumsel = np.zeros((128, 16), np.float32)
    for p in range(128):
        sumsel[p, (p // 32) * 4 + (p % 32) // 8] = 1.0

    xp = np.zeros((B, NC, LC, H + 4, WPAD), np.float32)
    xp[:, :, :, 2:-2, 2:-2] = x
    if USE_FP8X:
        import ml_dtypes
        xq = xp.astype(ml_dtypes.float8_e4m3)
    else:
        xq = xp.astype(NPF16)

    in_maps = []
    for k in range(NCORES):
        r0 = k * HB
        in_maps.append({
            "xs": np.ascontiguousarray(xq[:, :, :, r0:r0 + HB + 4, :]),
            "wt": wt, "selnp": selnp, "selb": selb, "sumsel": sumsel,
            "csel": csel,
        })
    return in_maps


def _get_rt():
    """Build (once) a cached jit'd shard_map runner over the 8 cores.

    run_bass_kernel_spmd constructs a fresh jax.jit closure per call (re-trace
    + compile every time) and ships host-side zero output buffers through the
    axon tunnel; this runner is built once and makes the donated output
    buffers on-device.
    """
    if "rt" in _cache:
        return _cache["rt"]
    import jax
    import jax.numpy as jnp
    from jax.sharding import Mesh, PartitionSpec, NamedSharding
    from jax.experimental.shard_map import shard_map
    from concourse import bass2jax

    bass2jax.install_neuronx_cc_hook()
    nc = _cache.get("nc")
    if nc is None:
        nc = _cache["nc"] = build_nc()
    partition_name = nc.partition_id_tensor.name if nc.partition_id_tensor else None

    in_names, out_names, out_avals = [], [], []
    for alloc in nc.m.functions[0].allocations:
        if not isinstance(alloc, mybir.MemoryLocationSet):
            continue
        name = alloc.memorylocations[0].name
        if alloc.kind == "ExternalInput":
            if name != partition_name:
                in_names.append(name)
        elif alloc.kind == "ExternalOutput":
            out_names.append(name)
            out_avals.append(jax.core.ShapedArray(
                tuple(alloc.tensor_shape), mybir.dt.np(alloc.dtype)))
    n_params, n_outs = len(in_names), len(out_names)
    all_in = tuple(in_names + out_names
                   + ([partition_name] if partition_name else []))

    devices = jax.devices()[:NCORES]
    mesh = Mesh(np.asarray(devices), ("core",))

    def _body(*args):
        operands = list(args)
        if partition_name is not None:
            operands.append(bass2jax.partition_id_tensor())
        return tuple(bass2jax._bass_exec_p.bind(
            *operands, out_avals=tuple(out_avals), in_names=all_in,
            out_names=tuple(out_names), lowering_input_output_aliases=(),
            sim_require_finite=True, sim_require_nnan=True, nc=nc))

    spec = PartitionSpec("core")
    sharded = jax.jit(
        shard_map(_body, mesh=mesh, in_specs=(spec,) * (n_params + n_outs),
                  out_specs=(spec,) * n_outs, check_rep=False),
        donate_argnums=tuple(range(n_params, n_params + n_outs)),
        keep_unused=True)

    zsh = NamedSharding(mesh, spec)

    def _mk_zf(shape, dtype):
        return jax.jit(lambda: jnp.zeros(shape, dtype), out_shardings=zsh)

    zfns = [_mk_zf((NCORES * a.shape[0],) + tuple(a.shape[1:]), a.dtype)
            for a in out_avals]

    def run(in_maps):
        per_core = [[np.asarray(m[nm]) for nm in in_names] for m in in_maps]
        concat_in = [np.concatenate([per_core[c][i] for c in range(NCORES)], axis=0)
                     for i in range(n_params)]
        zs = [zf() for zf in zfns]
        outs = sharded(*concat_in, *zs)
        return {nm: np.asarray(o) for nm, o in zip(out_names, outs)}

    _cache["rt"] = run
    return run


def kernel(x, W):
    in_maps = _prep_inputs(x, W)
    if axon_active():
        run = _get_rt()
        yg = run(in_maps)["y"]                      # [8*B, 128, HB, Wd] fp16
        y = yg.reshape(NCORES, B, 128, HB, Wd)
    else:
        from concourse.bass_utils import run_bass_kernel_spmd
        if "nc" not in _cache:
            _cache["nc"] = build_nc()
        res = run_bass_kernel_spmd(_cache["nc"], in_maps, list(range(NCORES))).results
        y = np.stack([r["y"] for r in res])
    out = y.transpose(1, 2, 0, 3, 4).reshape(B, 128, H, Wd)
    return out.reshape(B, NP, LP, H, Wd).astype(np.float32)
